# revision 5
# baseline (speedup 1.0000x reference)
"""Trainium2 Bass kernel for nn_AttentionFlow (T=8192, J=1024, D=256, 8 cores).

Reference math:
  w_c, w_q, w_m = w[:D], w[D:2D], w[2D:]
  S[t,j] = ctx@w_c [t] + q@w_q [j] + (ctx*w_m) @ q.T     [T, J]
  A = softmax_j(S);  c2q = A @ q                          [T, D]
  b = max_j S;       h = b @ ctx                          [D]
  G = [ctx, c2q, ctx*c2q, ctx*h]                          [T, 4D]

Sharding: rows (t) split across 8 cores, 1024 rows each. qwq = q@w_q ([J])
is precomputed host-side and passed as an input.

Per-core structure (bf16 matmuls, f32 PSUM):
  prep:    load Q/X; Qm = q*w_m; QmT + ctxT via PE transposes.
  phase B (per 128-col j-chunk): V.T = Qm @ ctx.T via matmuls;
           E.T = exp(V.T + qwq_j) via scalar activation (bias = per-
           partition qwq). exp(cwc_t) cancels in the softmax so it is
           left out of E.
  max:     b_t = cwc_t + ln(max_j E.T[j,t]).  The j-max is a DVE
           partition-halving tree (128->64->32 per chunk, then a pairwise
           jc tree), 8 small PE transposes, one reduce_max, one Ln.
  h:       ph = b @ ctx (8 tiny matmuls) -> broadcast to [128,256] via a
           K=1 ones matmul -> 3-round XOR recursive-doubling allreduce
           across the 8 cores via gpsimd remote_dma_broadcast (inside a
           tile_critical so only the Pool engine is gated).  A neutered
           bir_kernel_barrier keeps a real collective in the NEFF, which
           makes the runtime synchronize the 8 core launches; nothing
           ever waits on the slow ncfw op itself.
  phase C (per t-tile): U = E.T-chunks.T @ [q | 1]; c2q = U[:,:D]/U[:,D];
           emit [c2q, ctx*c2q]; block 1 (ctx) is DMAed straight from the
           input tile.
  phase D: G4 = ctx * h after the exchange lands; single 1MB DMA.
"""

import sys

if "/opt/trn_rl_repo" not in sys.path:
    sys.path.insert(0, "/opt/trn_rl_repo")

import numpy as np

import concourse.bass as bass
import concourse.bacc as bacc
import concourse.tile as tile
from concourse import bass_isa, mybir
from concourse.bass_utils import run_bass_kernel_spmd
from concourse.masks import make_identity

T, J, D = 8192, 1024, 256
N_CORES = 8
T_LOC = T // N_CORES          # 1024 rows per core
NT = T_LOC // 128             # 8 t-tiles per core
NJ = J // 128                 # 8 j-chunks
F32 = mybir.dt.float32
BF16 = mybir.dt.bfloat16


def _build_program():
    nc = bacc.Bacc("TRN2", target_bir_lowering=False, debug=False,
                   num_devices=N_CORES)
    ctx_ap = nc.dram_tensor("context", [T_LOC, D], F32, kind="ExternalInput").ap()
    q_ap = nc.dram_tensor("query", [J, D], F32, kind="ExternalInput").ap()
    w_ap = nc.dram_tensor("w", [3 * D], F32, kind="ExternalInput").ap()
    qwq_ap = nc.dram_tensor("qwq", [J], F32, kind="ExternalInput").ap()
    out_ap = nc.dram_tensor("out", [T_LOC, 4 * D], F32, kind="ExternalOutput").ap()

    with tile.TileContext(nc) as tc:
        _emit(tc, out_ap, ctx_ap, q_ap, w_ap, qwq_ap)
        tc._emit_exitstack.close()
    nc.compile()
    return nc


def _emit(tc, out_ap, ctx_ap, q_ap, w_ap, qwq_ap):
    from contextlib import ExitStack
    nc = tc.nc
    AF = mybir.ActivationFunctionType

    es = ExitStack()
    tc._emit_exitstack = es
    singles = es.enter_context(tc.tile_pool(name="singles", bufs=1))
    wk_g = es.enter_context(tc.tile_pool(name="wk_g", bufs=3))
    ps_S = es.enter_context(tc.tile_pool(name="ps_S", bufs=2, space="PSUM"))
    ps_TC = es.enter_context(tc.tile_pool(name="ps_TC", bufs=2, space="PSUM"))
    ps_U = es.enter_context(tc.tile_pool(name="ps_U", bufs=2, space="PSUM"))
    ps_M = es.enter_context(tc.tile_pool(name="ps_M", bufs=2, space="PSUM"))

    # ---------------- one-time prep ----------------
    ident = singles.tile([128, 128], BF16)
    make_identity(nc, ident)

    # query, natural layout [p, jc, d]  (j = jc*128 + p)
    q_f32 = singles.tile([128, NJ, D], F32)
    nc.sync.dma_start(out=q_f32, in_=q_ap.rearrange("(c p) d -> p c d", p=128))
    # context shard, natural layout [p, t, d]
    ctx_f32 = singles.tile([128, NT, D], F32)
    nc.sync.dma_start(out=ctx_f32, in_=ctx_ap.rearrange("(c p) d -> p c d", p=128))

    # w_m broadcast across partitions; qwq in partition-major column form
    wm_bc = singles.tile([128, D], F32)
    nc.sync.dma_start(
        out=wm_bc,
        in_=w_ap[2 * D:3 * D].rearrange("(a d) -> a d", a=1).to_broadcast([128, D]))
    qwqT = singles.tile([128, NJ], F32)
    nc.sync.dma_start(out=qwqT, in_=qwq_ap.rearrange("(c p) -> p c", p=128))
    ones_bf = singles.tile([1, 128], BF16)
    nc.vector.memset(ones_bf, 1.0)
    # w_c in partition-major form for the cwc matmuls
    wc_pm = singles.tile([128, 2], F32)
    nc.sync.dma_start(out=wc_pm, in_=w_ap[0:D].rearrange("(c p) -> p c", p=128))
    wc_pm_bf = singles.tile([128, 2], BF16)
    nc.scalar.copy(wc_pm_bf, wc_pm)

    # q_aug: bf16 [q | 1] moving operand of the U matmuls
    q_aug = singles.tile([128, NJ, D + 1], BF16)
    qm_bf = singles.tile([128, NJ, D], BF16)
    for jc in range(NJ):
        nc.scalar.copy(q_aug[:, jc, 0:D], q_f32[:, jc, :])
        nc.vector.tensor_mul(qm_bf[:, jc, :], q_f32[:, jc, :], wm_bc)
    nc.vector.memset(q_aug[:, :, D:D + 1], 1.0)

    # QmT [d-partitions, dc, j] via PE transposes
    QmT = singles.tile([128, 2, J], BF16)
    for jc in range(NJ):
        for dc in range(2):
            pt = ps_TC.tile([128, 128], BF16, tag="T")
            nc.tensor.transpose(pt, qm_bf[:, jc, dc * 128:(dc + 1) * 128], ident)
            nc.scalar.copy(QmT[:, dc, jc * 128:(jc + 1) * 128], pt)

    # ctx cast + transposes
    ctx_bf = singles.tile([128, NT, D], BF16)
    ctxT_all = singles.tile([128, 2, T_LOC], BF16)
    for t in range(NT):
        nc.scalar.copy(ctx_bf[:, t, :], ctx_f32[:, t, :])
        for dc in range(2):
            pt = ps_TC.tile([128, 128], BF16, tag="T")
            nc.tensor.transpose(pt, ctx_bf[:, t, dc * 128:(dc + 1) * 128], ident)
            nc.scalar.copy(ctxT_all[:, dc, t * 128:(t + 1) * 128], pt)

    # ---------------- phase B: E.T per j-chunk + running jc max -------------
    ET_all = singles.tile([128, NJ, T_LOC], BF16)
    mruns = [singles.tile([128, T_LOC], BF16, name=f"mrun{i}", tag=f"mrun{i % 2}")
             for i in range(NJ - 1)]
    for jc in range(NJ):
        for th in range(2):
            ps = ps_S.tile([128, 512], F32, tag="S")
            for dc in range(2):
                nc.tensor.matmul(
                    ps, QmT[:, dc, jc * 128:(jc + 1) * 128],
                    ctxT_all[:, dc, th * 512:(th + 1) * 512],
                    start=(dc == 0), stop=(dc == 1))
            nc.scalar.activation(ET_all[:, jc, th * 512:(th + 1) * 512], ps,
                                 AF.Exp, bias=qwqT[:, jc:jc + 1])
        # running elementwise max over j-chunks (free-axis, pipelined with B)
        if jc >= 1:
            prev = ET_all[:, 0, :] if jc == 1 else mruns[jc - 2]
            nc.vector.tensor_max(mruns[jc - 1], prev, ET_all[:, jc, :])
    e3 = mruns[NJ - 2]

    # cross-partition max of the remaining 128 j-rows (gpsimd); result is
    # replicated across all partitions, t still on the free axis
    em = singles.tile([128, T_LOC], BF16)
    nc.gpsimd.partition_all_reduce(em, e3, channels=128,
                                   reduce_op=bass_isa.ReduceOp.max)
    # transpose 128-blocks of em; every column of a transposed block equals
    # m[t] for that block's t-range -> gather column 0 into maxE [128, NT]
    ptT = ps_M.tile([128, NT, 128], BF16, tag="M")
    for t in range(NT):
        nc.tensor.transpose(ptT[:, t, :], em[:, t * 128:(t + 1) * 128], ident)
    maxE = singles.tile([128, NT], F32)
    nc.vector.tensor_copy(maxE, ptT[:, :, 0:1])
    lnm = singles.tile([128, NT], F32)
    nc.scalar.activation(lnm, maxE, AF.Ln)

    # cwc[t] = ctx @ w_c, accumulated per t-tile into one [128, NT] SBUF tile
    cwc = singles.tile([128, NT], F32)
    for t in range(NT):
        ps_c = ps_TC.tile([128, 1], F32, tag="T")
        for dc in range(2):
            nc.tensor.matmul(ps_c, ctxT_all[:, dc, t * 128:(t + 1) * 128],
                             wc_pm_bf[:, dc:dc + 1],
                             start=(dc == 0), stop=(dc == 1))
        nc.scalar.copy(cwc[:, t:t + 1], ps_c)
    b_bf = singles.tile([128, NT], BF16)
    nc.vector.tensor_add(b_bf, lnm, cwc)

    # ---------------- h partial + broadcast + cross-core exchange ----------
    ph = ps_M.tile([1, D], F32, tag="M")
    for t in range(NT):
        nc.tensor.matmul(ph, b_bf[:, t:t + 1], ctx_bf[:, t, :],
                         start=(t == 0), stop=(t == NT - 1),
                         skip_group_check=True)
    ph_sb = singles.tile([1, D], BF16)
    nc.scalar.copy(ph_sb, ph)
    hb_ps = ps_M.tile([128, D], F32, tag="M")
    nc.tensor.matmul(hb_ps, ones_bf, ph_sb, start=True, stop=True)
    hx0 = singles.tile([128, D], F32)
    nc.vector.tensor_copy(hx0, hb_ps)

    # 3-round XOR recursive-doubling allreduce(sum) over the 8 cores.
    psem = nc.alloc_semaphore("x_psem")
    asem = nc.alloc_semaphore("x_asem")
    lsem = nc.alloc_semaphore("x_lsem")
    rsems = [nc.alloc_semaphore(f"x_rsem{r}") for r in range(3)]
    recvs, accs = [], [hx0]
    for rnd in range(3):
        r_t = singles.tile([128, D], F32, name=f"x_recv{rnd}")
        a_t = singles.tile([128, D], F32, name=f"x_acc{rnd}")
        recvs.append(r_t)
        accs.append(a_t)
    with tc.tile_critical():
        for rnd, delta in enumerate((1, 2, 4)):
            recv, acc, nxt = recvs[rnd], accs[rnd], accs[rnd + 1]
            rdests = [None] * 8
            rdests[4 if delta & 4 else 0] = (0, delta)
            nc.gpsimd.remote_dma_broadcast(
                out_ap=recv[:, :], in_ap=acc[:, :],
                remote_sem=rsems[rnd], local_sem=lsem,
                rdests=rdests).then_inc(psem, 1)
            nc.gpsimd.wait_ge(psem, rnd + 1)
            if rnd > 0:
                nc.gpsimd.wait_ge(asem, rnd)
            nc.gpsimd.trigger_dma(count=1)
            nc.gpsimd.tensor_add(nxt, acc, recv)._wait_ge(
                rsems[rnd], 2).then_inc(asem, 1)
        # Decoy collective: keeps a real multi-core CC op in the NEFF so the
        # runtime synchronizes core launches.  The wait itself is neutered
        # (value 0) so nothing stalls on the slow ncfw op.
        _bw = nc.gpsimd.bir_kernel_barrier_wait(
            [[0, 1, 2, 3, 4, 5, 6, 7]])
        for _w in _bw.ins.sync_info.on_wait:
            _w.wait_value = 0
    hsum = accs[-1]

    # ---------------- block 1: ctx verbatim, straight from SBUF -----------
    nc.sync.dma_start(
        out=out_ap[:, 0:D].rearrange("(c p) d -> p c d", p=128), in_=ctx_f32)

    # ---------------- phase C: U, c2q, G[:, D:3D] per t-tile ----------------
    for t in range(NT):
        rows = slice(t * 128, (t + 1) * 128)
        pu = ps_U.tile([128, D + 1], F32, tag="U")
        for jc in range(NJ):
            nc.tensor.matmul(pu, ET_all[:, jc, t * 128:(t + 1) * 128],
                             q_aug[:, jc, :],
                             start=(jc == 0), stop=(jc == NJ - 1))
        r = wk_g.tile([128, 1], F32, tag="recip")
        nc.vector.reciprocal(r, pu[:, D:D + 1])
        g = wk_g.tile([128, 2 * D], F32, tag="g")
        nc.scalar.activation(g[:, 0:D], pu[:, 0:D], AF.Copy, scale=r)
        nc.vector.tensor_mul(g[:, D:2 * D], ctx_f32[:, t, :], g[:, 0:D])
        nc.scalar.dma_start(out=out_ap[rows, D:3 * D], in_=g)

    # ---------------- phase D: G4 after the exchange ----------------
    g4 = singles.tile([128, NT, D], F32)
    for t in range(NT):
        nc.vector.tensor_mul(g4[:, t, :], ctx_f32[:, t, :], hsum)
    nc.sync.dma_start(
        out=out_ap[:, 3 * D:4 * D].rearrange("(c p) d -> p c d", p=128), in_=g4)


_NC_CACHE = None


def _get_program():
    global _NC_CACHE
    if _NC_CACHE is None:
        _NC_CACHE = _build_program()
    return _NC_CACHE


def kernel(context: np.ndarray, query: np.ndarray, w: np.ndarray,
           **kwargs) -> np.ndarray:
    context = np.ascontiguousarray(context, dtype=np.float32)
    query = np.ascontiguousarray(query, dtype=np.float32)
    w = np.ascontiguousarray(w, dtype=np.float32)
    qwq = query @ w[D:2 * D]

    nc = _get_program()
    shard = T_LOC
    in_maps = [
        {
            "context": context[i * shard:(i + 1) * shard],
            "query": query,
            "w": w,
            "qwq": qwq,
        }
        for i in range(N_CORES)
    ]
    res = run_bass_kernel_spmd(nc, in_maps, core_ids=list(range(N_CORES)))
    return np.concatenate([res.results[i]["out"] for i in range(N_CORES)],
                          axis=0)
